# revision 1
# baseline (speedup 1.0000x reference)
# MoE (top-2 of 8 experts) Trainium2 kernel.
#
# Strategy (expert-parallel, matches the sharding hint):
#   - Gate (softmax + top-2 + renormalize) computed on host in f32 — it is
#     0.006% of the FLOPs and produces the data-dependent routing needed to
#     shard the tokens.
#   - Token dispatch = the host-side sharding step: tokens routed to expert e
#     are gathered (transposed, bf16-cast, padded to a uniform capacity) and
#     sent to core e together with expert e's weights.
#   - Each core runs a dense FFN  relu(x @ w1 + b1) @ w2 + b2  over its token
#     batch on the TensorEngine (bf16 inputs, fp32 PSUM accumulation).
#   - Combine = host-side unshard: out[tok] += gate_weight * y_core[tok].
#
# Device kernel layout (all "transposed": tokens on the matmul free dim):
#   phase 1:  hT[mh]  = relu( w1[kc,:,mh*128:..].T @ xT[kc]  summed over kc + b1 )
#   phase 2:  outT[mc] =       w2[kh,:,mc*128:..].T @ hT[kh]  summed over kh + b2
# w1/w2 stay resident in SBUF (bf16, 16 MB); token chunks of 512 stream through.

import os
import sys
import types

import numpy as np
import ml_dtypes

P = 128
C = 1024
H = 4096
E = 8
N_CORES = 8
KC = C // P   # 8
KH = H // P   # 32
BF16 = ml_dtypes.bfloat16

TRACE = bool(int(os.environ.get("KERNEL_TRACE", "0")))
LAST_EXEC_NS = None
LAST_RESULTS = None


def _ensure_axon_hooks_shim():
    """bass_utils imports antenv.axon_hooks when tracing is requested; this
    image's antenv lacks that module. Provide it, backed by the axon PJRT .so
    profiling C ABI when available."""
    try:
        import antenv.axon_hooks  # noqa: F401
        return
    except ImportError:
        pass
    mod = types.ModuleType("antenv.axon_hooks")
    mod._hook = None

    def set_axon_ntff_profile_hook(h):
        mod._hook = h

    def get_axon_ntff_profile_hook():
        return mod._hook

    mod.set_axon_ntff_profile_hook = set_axon_ntff_profile_hook
    mod.get_axon_ntff_profile_hook = get_axon_ntff_profile_hook
    try:
        import antenv
        sys.modules["antenv.axon_hooks"] = mod
        antenv.axon_hooks = mod
    except ImportError:
        antenv = types.ModuleType("antenv")
        antenv.axon_hooks = mod
        sys.modules["antenv"] = antenv
        sys.modules["antenv.axon_hooks"] = mod
    try:
        from trn_agent_boot.trn_boot import _ntff_profile_via_ctypes
        h = _ntff_profile_via_ctypes("/opt/axon/libaxon_pjrt.so")
        if h is not None:
            mod._hook = h
    except Exception:
        pass


_COMPILED = {}


def _build(cap, chunks):
    import concourse.mybir as mybir
    import concourse.tile as tile
    from concourse import bacc

    f32 = mybir.dt.float32
    bf16 = mybir.dt.bfloat16

    nc = bacc.Bacc("TRN2", target_bir_lowering=False, debug=False,
                   num_devices=N_CORES)

    xt_d = nc.dram_tensor("xt", [C, cap], bf16, kind="ExternalInput")
    w1_d = nc.dram_tensor("w1t", [C, H], bf16, kind="ExternalInput")
    w2_d = nc.dram_tensor("w2t", [H, C], bf16, kind="ExternalInput")
    b1_d = nc.dram_tensor("b1r", [P, KH], f32, kind="ExternalInput")
    b2_d = nc.dram_tensor("b2r", [P, KC], f32, kind="ExternalInput")
    out_d = nc.dram_tensor("out", [C, cap], f32, kind="ExternalOutput")

    xt_t = xt_d.ap().rearrange("(kc p) n -> kc p n", p=P)
    w1_t = w1_d.ap().rearrange("(kc p) h -> kc p h", p=P)
    w2_t = w2_d.ap().rearrange("(kh p) c -> kh p c", p=P)
    out_t = out_d.ap().rearrange("(mc p) n -> mc p n", p=P)

    relu = mybir.ActivationFunctionType.Relu

    with tile.TileContext(nc) as tc:
        with (
            tc.tile_pool(name="wres", bufs=1) as wpool,
            tc.tile_pool(name="bias", bufs=1) as bpool,
            tc.tile_pool(name="xin", bufs=1) as xpool,
            tc.tile_pool(name="hmid", bufs=1) as hpool,
            tc.tile_pool(name="oout", bufs=1) as opool,
            tc.tile_pool(name="ps1", bufs=4, space="PSUM") as ps1pool,
            tc.tile_pool(name="ps2", bufs=4, space="PSUM") as ps2pool,
        ):
            w1_sb = []
            for kc in range(KC):
                t = wpool.tile([P, H], bf16, tag=f"w1_{kc}")
                nc.sync.dma_start(t[:], w1_t[kc])
                w1_sb.append(t)
            w2_sb = []
            for kh in range(KH):
                t = wpool.tile([P, C], bf16, tag=f"w2_{kh}")
                nc.sync.dma_start(t[:], w2_t[kh])
                w2_sb.append(t)
            b1_sb = bpool.tile([P, KH], f32, tag="b1")
            nc.sync.dma_start(b1_sb[:], b1_d.ap())
            b2_sb = bpool.tile([P, KC], f32, tag="b2")
            nc.sync.dma_start(b2_sb[:], b2_d.ap())

            off = 0
            for W in chunks:
                x_sb = []
                for kc in range(KC):
                    t = xpool.tile([P, W], bf16, tag=f"x_{kc}")
                    nc.sync.dma_start(t[:], xt_t[kc][:, off:off + W])
                    x_sb.append(t)

                h_sb = []
                for mh in range(KH):
                    ps = ps1pool.tile([P, W], f32, tag="ps1")
                    for kc in range(KC):
                        nc.tensor.matmul(
                            ps[:],
                            w1_sb[kc][:, mh * P:(mh + 1) * P],
                            x_sb[kc][:],
                            start=(kc == 0),
                            stop=(kc == KC - 1),
                        )
                    ht = hpool.tile([P, W], bf16, tag=f"h_{mh}")
                    nc.scalar.activation(ht[:], ps[:], relu,
                                         bias=b1_sb[:, mh:mh + 1], scale=1.0)
                    h_sb.append(ht)

                for mc in range(KC):
                    ps = ps2pool.tile([P, W], f32, tag="ps2")
                    for kh in range(KH):
                        nc.tensor.matmul(
                            ps[:],
                            w2_sb[kh][:, mc * P:(mc + 1) * P],
                            h_sb[kh][:],
                            start=(kh == 0),
                            stop=(kh == KH - 1),
                        )
                    ot = opool.tile([P, W], f32, tag=f"o_{mc}")
                    nc.vector.tensor_scalar_add(ot[:], ps[:],
                                                b2_sb[:, mc:mc + 1])
                    nc.sync.dma_start(out_t[mc][:, off:off + W], ot[:])
                off += W

    nc.compile()
    return nc


def _get_compiled(cap, chunks):
    key = (cap, tuple(chunks))
    if key not in _COMPILED:
        _COMPILED[key] = _build(cap, list(chunks))
    return _COMPILED[key]


def kernel(x, gate_w, w1, b1, w2, b2):
    global LAST_EXEC_NS, LAST_RESULTS
    _ensure_axon_hooks_shim()
    from concourse import bass_utils

    B, T, _ = x.shape
    N = B * T
    xf = np.ascontiguousarray(x.reshape(N, C)).astype(np.float32, copy=False)

    # --- gate on host (f32, matches reference numerics) ---
    logits = xf @ np.ascontiguousarray(gate_w.astype(np.float32)).T
    m = logits.max(axis=1, keepdims=True)
    ew = np.exp(logits - m)
    sw = ew / ew.sum(axis=1, keepdims=True)        # [N, E] f32 softmax
    ar = np.arange(N)
    i0 = sw.argmax(axis=1)
    w0 = sw[ar, i0]
    swm = sw.copy()
    swm[ar, i0] = -1.0
    i1 = swm.argmax(axis=1)
    w1g = sw[ar, i1]
    tot = w0 + w1g
    cw0 = (w0 / tot).astype(np.float32)
    cw1 = (w1g / tot).astype(np.float32)

    # --- dispatch: token lists per expert ---
    idx_list, cw_list = [], []
    for e in range(E):
        s0 = i0 == e
        s1 = i1 == e
        idx_list.append(np.concatenate([ar[s0], ar[s1]]))
        cw_list.append(np.concatenate([cw0[s0], cw1[s1]]).astype(np.float32))
    counts = [len(ix) for ix in idx_list]
    cap = ((max(counts) + P - 1) // P) * P
    chunks = [512] * (cap // 512)
    if cap % 512:
        chunks.append(cap % 512)

    nc = _get_compiled(cap, chunks)

    # --- per-core inputs ---
    w1b = w1.astype(BF16)                                        # [E, C, H]
    w2b = w2.astype(BF16)                                        # [E, H, C]
    b1r = np.ascontiguousarray(
        b1.astype(np.float32).reshape(E, KH, P).transpose(0, 2, 1))
    b2r = np.ascontiguousarray(
        b2.astype(np.float32).reshape(E, KC, P).transpose(0, 2, 1))
    in_maps = []
    for e in range(E):
        xt = np.zeros((C, cap), dtype=BF16)
        xt[:, :counts[e]] = np.ascontiguousarray(xf[idx_list[e]].T)
        in_maps.append({
            "xt": xt,
            "w1t": np.ascontiguousarray(w1b[e]),
            "w2t": np.ascontiguousarray(w2b[e]),
            "b1r": b1r[e],
            "b2r": b2r[e],
        })

    res = bass_utils.run_bass_kernel_spmd(
        nc, in_maps, core_ids=list(range(N_CORES)), trace=TRACE)
    LAST_RESULTS = res
    LAST_EXEC_NS = res.exec_time_ns

    # --- combine (host unshard) ---
    out = np.zeros((N, C), dtype=np.float32)
    for e in range(E):
        n_e = counts[e]
        y = res.results[e]["out"][:, :n_e].T                     # [n_e, C] f32
        out[idx_list[e]] += cw_list[e][:, None] * y
    return out.reshape(B, T, C).astype(x.dtype, copy=False)


# revision 3
# speedup vs baseline: 1.0100x; 1.0100x over previous
# MoE (top-2 of 8 experts) Trainium2 kernel.
#
# Strategy (expert-parallel, matches the sharding hint):
#   - Gate (softmax + top-2 + renormalize) computed on host in f32 — it is
#     0.006% of the FLOPs and produces the data-dependent routing needed to
#     shard the tokens.
#   - Token dispatch = the host-side sharding step: tokens routed to expert e
#     are gathered (transposed, bf16-cast, padded to a uniform capacity) and
#     sent to core e together with expert e's weights.
#   - Each core runs a dense FFN  relu(x @ w1 + b1) @ w2 + b2  over its token
#     batch on the TensorEngine (bf16 inputs, fp32 PSUM accumulation).
#   - Combine = host-side unshard: out[tok] += gate_weight * y_core[tok].
#
# Device kernel layout (all "transposed": tokens on the matmul free dim):
#   phase 1:  hT[mh]  = relu( w1[kc,:,mh*128:..].T @ xT[kc]  summed over kc + b1 )
#   phase 2:  outT[mc] =       w2[kh,:,mc*128:..].T @ hT[kh]  summed over kh + b2
# w1/w2 stay resident in SBUF (bf16, 16 MB); token chunks of 512 stream through.

import os
import sys
import types

import numpy as np
import ml_dtypes

P = 128
C = 1024
H = 4096
E = 8
N_CORES = 8
KC = C // P   # 8
KH = H // P   # 32
BF16 = ml_dtypes.bfloat16

TRACE = bool(int(os.environ.get("KERNEL_TRACE", "0")))
LAST_EXEC_NS = None
LAST_RESULTS = None


def _ensure_axon_hooks_shim():
    """bass_utils imports antenv.axon_hooks when tracing is requested; this
    image's antenv lacks that module. Provide it, backed by the axon PJRT .so
    profiling C ABI when available."""
    try:
        import antenv.axon_hooks  # noqa: F401
        return
    except ImportError:
        pass
    mod = types.ModuleType("antenv.axon_hooks")
    mod._hook = None

    def set_axon_ntff_profile_hook(h):
        mod._hook = h

    def get_axon_ntff_profile_hook():
        return mod._hook

    mod.set_axon_ntff_profile_hook = set_axon_ntff_profile_hook
    mod.get_axon_ntff_profile_hook = get_axon_ntff_profile_hook
    try:
        import antenv
        sys.modules["antenv.axon_hooks"] = mod
        antenv.axon_hooks = mod
    except ImportError:
        antenv = types.ModuleType("antenv")
        antenv.axon_hooks = mod
        sys.modules["antenv"] = antenv
        sys.modules["antenv.axon_hooks"] = mod
    try:
        from trn_agent_boot.trn_boot import _ntff_profile_via_ctypes
        h = _ntff_profile_via_ctypes("/opt/axon/libaxon_pjrt.so")
        if h is not None:
            mod._hook = h
    except Exception:
        pass


_COMPILED = {}


def _build(cap, chunks):
    import concourse.mybir as mybir
    import concourse.tile as tile
    from concourse import bacc

    f32 = mybir.dt.float32
    bf16 = mybir.dt.bfloat16

    nc = bacc.Bacc("TRN2", target_bir_lowering=False, debug=False,
                   num_devices=N_CORES)

    xt_d = nc.dram_tensor("xt", [C, cap], bf16, kind="ExternalInput")
    w1_d = nc.dram_tensor("w1t", [C, H], bf16, kind="ExternalInput")
    w2_d = nc.dram_tensor("w2t", [H, C], bf16, kind="ExternalInput")
    b1_d = nc.dram_tensor("b1r", [P, KH], f32, kind="ExternalInput")
    b2_d = nc.dram_tensor("b2r", [P, KC], f32, kind="ExternalInput")
    out_d = nc.dram_tensor("out", [C, cap], f32, kind="ExternalOutput")

    xt_t = xt_d.ap().rearrange("(kc p) n -> kc p n", p=P)
    w1_t = w1_d.ap().rearrange("(kc p) h -> kc p h", p=P)
    w2_t = w2_d.ap().rearrange("(kh p) c -> kh p c", p=P)
    out_t = out_d.ap().rearrange("(mc p) n -> mc p n", p=P)

    relu = mybir.ActivationFunctionType.Relu

    # w1 is loaded in column blocks so the first phase-1 groups can start
    # after ~2 MB of DMA instead of waiting for all 16 MB of weights.
    W1BLK = 4                 # column blocks per w1 kc-tile
    W1BW = H // W1BLK         # 1024 columns per block
    GRP_PER_BLK = W1BW // P   # 8 mh-groups per block

    with tile.TileContext(nc) as tc:
        with (
            tc.tile_pool(name="wres", bufs=1) as wpool,
            tc.tile_pool(name="bias", bufs=1) as bpool,
            tc.tile_pool(name="xin", bufs=1) as xpool,
            tc.tile_pool(name="hmid", bufs=1) as hpool,
            tc.tile_pool(name="oout", bufs=1) as opool,
            tc.tile_pool(name="ps1", bufs=4, space="PSUM") as ps1pool,
            tc.tile_pool(name="ps2", bufs=4, space="PSUM") as ps2pool,
        ):
            # first token chunk before weights: it is on the critical path
            x_first = []
            for kc in range(KC):
                t = xpool.tile([P, chunks[0]], bf16, tag=f"x_{kc}")
                nc.sync.dma_start(t[:], xt_t[kc][:, 0:chunks[0]])
                x_first.append(t)

            w1_sb = [[None] * W1BLK for _ in range(KC)]
            for blk in range(W1BLK):
                for kc in range(KC):
                    t = wpool.tile([P, W1BW], bf16, tag=f"w1_{kc}_{blk}")
                    nc.sync.dma_start(
                        t[:], w1_t[kc][:, blk * W1BW:(blk + 1) * W1BW])
                    w1_sb[kc][blk] = t
            w2_sb = []
            for kh in range(KH):
                t = wpool.tile([P, C], bf16, tag=f"w2_{kh}")
                nc.sync.dma_start(t[:], w2_t[kh])
                w2_sb.append(t)
            b1_sb = bpool.tile([P, KH], f32, tag="b1")
            nc.sync.dma_start(b1_sb[:], b1_d.ap())
            b2_sb = bpool.tile([P, KC], f32, tag="b2")
            nc.sync.dma_start(b2_sb[:], b2_d.ap())

            off = 0
            for ci, W in enumerate(chunks):
                if ci == 0:
                    x_sb = x_first
                else:
                    x_sb = []
                    for kc in range(KC):
                        t = xpool.tile([P, W], bf16, tag=f"x_{kc}")
                        nc.sync.dma_start(t[:], xt_t[kc][:, off:off + W])
                        x_sb.append(t)

                h_sb = []
                for mh in range(KH):
                    blk, col = divmod(mh, GRP_PER_BLK)
                    ps = ps1pool.tile([P, W], f32, tag="ps1")
                    for kc in range(KC):
                        nc.tensor.matmul(
                            ps[:],
                            w1_sb[kc][blk][:, col * P:(col + 1) * P],
                            x_sb[kc][:],
                            start=(kc == 0),
                            stop=(kc == KC - 1),
                        )
                    ht = hpool.tile([P, W], bf16, tag=f"h_{mh}")
                    nc.scalar.activation(ht[:], ps[:], relu,
                                         bias=b1_sb[:, mh:mh + 1], scale=1.0)
                    h_sb.append(ht)

                for mc in range(KC):
                    ps = ps2pool.tile([P, W], f32, tag="ps2")
                    for kh in range(KH):
                        nc.tensor.matmul(
                            ps[:],
                            w2_sb[kh][:, mc * P:(mc + 1) * P],
                            h_sb[kh][:],
                            start=(kh == 0),
                            stop=(kh == KH - 1),
                        )
                    ot = opool.tile([P, W], f32, tag=f"o_{mc}")
                    nc.vector.tensor_scalar_add(ot[:], ps[:],
                                                b2_sb[:, mc:mc + 1])
                    nc.sync.dma_start(out_t[mc][:, off:off + W], ot[:])
                off += W

    nc.compile()
    return nc


def _get_compiled(cap, chunks):
    key = (cap, tuple(chunks))
    if key not in _COMPILED:
        _COMPILED[key] = _build(cap, list(chunks))
    return _COMPILED[key]


def kernel(x, gate_w, w1, b1, w2, b2):
    global LAST_EXEC_NS, LAST_RESULTS
    _ensure_axon_hooks_shim()
    from concourse import bass_utils

    B, T, _ = x.shape
    N = B * T
    xf = np.ascontiguousarray(x.reshape(N, C)).astype(np.float32, copy=False)

    # --- gate on host (f32, matches reference numerics) ---
    logits = xf @ np.ascontiguousarray(gate_w.astype(np.float32)).T
    m = logits.max(axis=1, keepdims=True)
    ew = np.exp(logits - m)
    sw = ew / ew.sum(axis=1, keepdims=True)        # [N, E] f32 softmax
    ar = np.arange(N)
    i0 = sw.argmax(axis=1)
    w0 = sw[ar, i0]
    swm = sw.copy()
    swm[ar, i0] = -1.0
    i1 = swm.argmax(axis=1)
    w1g = sw[ar, i1]
    tot = w0 + w1g
    cw0 = (w0 / tot).astype(np.float32)
    cw1 = (w1g / tot).astype(np.float32)

    # --- dispatch: token lists per expert ---
    idx_list, cw_list = [], []
    for e in range(E):
        s0 = i0 == e
        s1 = i1 == e
        idx_list.append(np.concatenate([ar[s0], ar[s1]]))
        cw_list.append(np.concatenate([cw0[s0], cw1[s1]]).astype(np.float32))
    counts = [len(ix) for ix in idx_list]
    cap = ((max(counts) + P - 1) // P) * P
    # equal-width chunks (multiples of 128, each <= 512): avoids a narrow
    # tail chunk where per-matmul LDWEIGHTS overhead is exposed
    n_chunks = -(-cap // 512)
    q, r = divmod(cap // P, n_chunks)
    chunks = [(q + 1) * P] * r + [q * P] * (n_chunks - r)

    nc = _get_compiled(cap, chunks)

    # --- per-core inputs ---
    w1b = w1.astype(BF16)                                        # [E, C, H]
    w2b = w2.astype(BF16)                                        # [E, H, C]
    b1r = np.ascontiguousarray(
        b1.astype(np.float32).reshape(E, KH, P).transpose(0, 2, 1))
    b2r = np.ascontiguousarray(
        b2.astype(np.float32).reshape(E, KC, P).transpose(0, 2, 1))
    in_maps = []
    for e in range(E):
        xt = np.zeros((C, cap), dtype=BF16)
        xt[:, :counts[e]] = np.ascontiguousarray(xf[idx_list[e]].T)
        in_maps.append({
            "xt": xt,
            "w1t": np.ascontiguousarray(w1b[e]),
            "w2t": np.ascontiguousarray(w2b[e]),
            "b1r": b1r[e],
            "b2r": b2r[e],
        })

    res = bass_utils.run_bass_kernel_spmd(
        nc, in_maps, core_ids=list(range(N_CORES)), trace=TRACE)
    LAST_RESULTS = res
    LAST_EXEC_NS = res.exec_time_ns

    # --- combine (host unshard) ---
    out = np.zeros((N, C), dtype=np.float32)
    for e in range(E):
        n_e = counts[e]
        y = res.results[e]["out"][:, :n_e].T                     # [n_e, C] f32
        out[idx_list[e]] += cw_list[e][:, None] * y
    return out.reshape(B, T, C).astype(x.dtype, copy=False)


# revision 4
# speedup vs baseline: 1.0752x; 1.0646x over previous
# MoE (top-2 of 8 experts) Trainium2 kernel.
#
# Strategy (expert-parallel, matches the sharding hint):
#   - Gate (softmax + top-2 + renormalize) computed on host in f32 — it is
#     0.006% of the FLOPs and produces the data-dependent routing needed to
#     shard the tokens.
#   - Token dispatch = the host-side sharding step: tokens routed to expert e
#     are gathered (transposed, bf16-cast, padded to a uniform capacity) and
#     sent to core e together with expert e's weights.
#   - Each core runs a dense FFN  relu(x @ w1 + b1) @ w2 + b2  over its token
#     batch on the TensorEngine (bf16 inputs, fp32 PSUM accumulation).
#   - Combine = host-side unshard: out[tok] += gate_weight * y_core[tok].
#
# Device kernel layout (all "transposed": tokens on the matmul free dim):
#   phase 1:  hT[mh]  = relu( w1[kc,:,mh*128:..].T @ xT[kc]  summed over kc + b1 )
#   phase 2:  outT[mc] =       w2[kh,:,mc*128:..].T @ hT[kh]  summed over kh + b2
# w1/w2 stay resident in SBUF (bf16, 16 MB); token chunks of 512 stream through.

import os
import sys
import types

import numpy as np
import ml_dtypes

P = 128
C = 1024
H = 4096
E = 8
N_CORES = 8
KC = C // P   # 8
KH = H // P   # 32
BF16 = ml_dtypes.bfloat16

TRACE = bool(int(os.environ.get("KERNEL_TRACE", "0")))
LAST_EXEC_NS = None
LAST_RESULTS = None


def _ensure_axon_hooks_shim():
    """bass_utils imports antenv.axon_hooks when tracing is requested; this
    image's antenv lacks that module. Provide it, backed by the axon PJRT .so
    profiling C ABI when available."""
    try:
        import antenv.axon_hooks  # noqa: F401
        return
    except ImportError:
        pass
    mod = types.ModuleType("antenv.axon_hooks")
    mod._hook = None

    def set_axon_ntff_profile_hook(h):
        mod._hook = h

    def get_axon_ntff_profile_hook():
        return mod._hook

    mod.set_axon_ntff_profile_hook = set_axon_ntff_profile_hook
    mod.get_axon_ntff_profile_hook = get_axon_ntff_profile_hook
    try:
        import antenv
        sys.modules["antenv.axon_hooks"] = mod
        antenv.axon_hooks = mod
    except ImportError:
        antenv = types.ModuleType("antenv")
        antenv.axon_hooks = mod
        sys.modules["antenv"] = antenv
        sys.modules["antenv.axon_hooks"] = mod
    try:
        from trn_agent_boot.trn_boot import _ntff_profile_via_ctypes
        h = _ntff_profile_via_ctypes("/opt/axon/libaxon_pjrt.so")
        if h is not None:
            mod._hook = h
    except Exception:
        pass


_COMPILED = {}


def _build(cap, chunks):
    import concourse.mybir as mybir
    import concourse.tile as tile
    from concourse import bacc

    f32 = mybir.dt.float32
    bf16 = mybir.dt.bfloat16

    nc = bacc.Bacc("TRN2", target_bir_lowering=False, debug=False,
                   num_devices=N_CORES)

    xt_d = nc.dram_tensor("xt", [C, cap], bf16, kind="ExternalInput")
    w1_d = nc.dram_tensor("w1t", [C, H], bf16, kind="ExternalInput")
    w2_d = nc.dram_tensor("w2t", [H, C], bf16, kind="ExternalInput")
    b1_d = nc.dram_tensor("b1r", [P, KH], f32, kind="ExternalInput")
    b2_d = nc.dram_tensor("b2r", [P, KC], f32, kind="ExternalInput")
    out_d = nc.dram_tensor("out", [C, cap], f32, kind="ExternalOutput")

    # partition-major views: [p, kc/kh/mc, free] so one DMA covers all
    # 128-row tiles of a tensor (each dma_start trigger costs ~600ns on the
    # Sync sequencer — merged transfers keep the trigger count tiny)
    xt_t = xt_d.ap().rearrange("(kc p) n -> p kc n", p=P)
    w1_t = w1_d.ap().rearrange("(kc p) h -> p kc h", p=P)
    w2_t = w2_d.ap().rearrange("(kh p) c -> p kh c", p=P)
    out_t = out_d.ap().rearrange("(mc p) n -> p mc n", p=P)

    relu = mybir.ActivationFunctionType.Relu

    # w1 is loaded in column blocks so the first phase-1 groups can start
    # after ~2 MB of DMA instead of waiting for all 16 MB of weights.
    W1BLK = 4                 # column blocks of w1
    W1BW = H // W1BLK         # 1024 columns per block
    GRP_PER_BLK = W1BW // P   # 8 mh-groups per block

    with tile.TileContext(nc) as tc:
        with (
            tc.tile_pool(name="wres", bufs=1) as wpool,
            tc.tile_pool(name="bias", bufs=1) as bpool,
            tc.tile_pool(name="xin", bufs=1) as xpool,
            tc.tile_pool(name="hmid", bufs=1) as hpool,
            tc.tile_pool(name="oout", bufs=1) as opool,
            tc.tile_pool(name="ps1", bufs=4, space="PSUM") as ps1pool,
            tc.tile_pool(name="ps2", bufs=4, space="PSUM") as ps2pool,
        ):
            # biases first: the phase-1 relu (which drains PSUM slots) needs
            # b1 — if it queued behind the weights, PE would stall on PSUM
            b1_sb = bpool.tile([P, KH], f32, tag="b1")
            nc.sync.dma_start(b1_sb[:], b1_d.ap())
            b2_sb = bpool.tile([P, KC], f32, tag="b2")
            nc.sync.dma_start(b2_sb[:], b2_d.ap())

            # first token chunk next: it is on the critical path
            W0 = chunks[0]
            x_first = xpool.tile([P, KC * W0], bf16, tag="x")
            nc.sync.dma_start(
                x_first[:].rearrange("p (kc w) -> p kc w", kc=KC),
                xt_t[:, :, 0:W0])

            # w1: one DMA per column block, [p, kc, W1BW] layout in SBUF
            w1_sb = []
            for blk in range(W1BLK):
                t = wpool.tile([P, KC * W1BW], bf16, tag=f"w1_{blk}")
                nc.sync.dma_start(
                    t[:].rearrange("p (kc w) -> p kc w", kc=KC),
                    w1_t[:, :, blk * W1BW:(blk + 1) * W1BW])
                w1_sb.append(t)
            # w2: one DMA per output-column block, [p, kh, P] layout
            w2_sb = []
            for mc in range(KC):
                t = wpool.tile([P, KH * P], bf16, tag=f"w2_{mc}")
                nc.sync.dma_start(
                    t[:].rearrange("p (kh w) -> p kh w", kh=KH),
                    w2_t[:, :, mc * P:(mc + 1) * P])
                w2_sb.append(t)

            off = 0
            for ci, W in enumerate(chunks):
                if ci == 0:
                    x_sb = x_first
                else:
                    x_sb = xpool.tile([P, KC * W], bf16, tag="x")
                    nc.sync.dma_start(
                        x_sb[:].rearrange("p (kc w) -> p kc w", kc=KC),
                        xt_t[:, :, off:off + W])

                h_sb = []
                for mh in range(KH):
                    blk, col = divmod(mh, GRP_PER_BLK)
                    ps = ps1pool.tile([P, W], f32, tag="ps1")
                    for kc in range(KC):
                        nc.tensor.matmul(
                            ps[:],
                            w1_sb[blk][:, (kc * GRP_PER_BLK + col) * P:
                                       (kc * GRP_PER_BLK + col) * P + P],
                            x_sb[:, kc * W:(kc + 1) * W],
                            start=(kc == 0),
                            stop=(kc == KC - 1),
                        )
                    ht = hpool.tile([P, W], bf16, tag=f"h_{mh}")
                    nc.scalar.activation(ht[:], ps[:], relu,
                                         bias=b1_sb[:, mh:mh + 1], scale=1.0)
                    h_sb.append(ht)

                o_sb = opool.tile([P, KC * W], f32, tag="o")
                for mc in range(KC):
                    ps = ps2pool.tile([P, W], f32, tag="ps2")
                    for kh in range(KH):
                        nc.tensor.matmul(
                            ps[:],
                            w2_sb[mc][:, kh * P:(kh + 1) * P],
                            h_sb[kh][:],
                            start=(kh == 0),
                            stop=(kh == KH - 1),
                        )
                    nc.vector.tensor_scalar_add(
                        o_sb[:, mc * W:(mc + 1) * W], ps[:],
                        b2_sb[:, mc:mc + 1])
                nc.sync.dma_start(
                    out_t[:, :, off:off + W],
                    o_sb[:].rearrange("p (mc w) -> p mc w", mc=KC))
                off += W

    nc.compile()
    return nc


def _get_compiled(cap, chunks):
    key = (cap, tuple(chunks))
    if key not in _COMPILED:
        _COMPILED[key] = _build(cap, list(chunks))
    return _COMPILED[key]


def kernel(x, gate_w, w1, b1, w2, b2):
    global LAST_EXEC_NS, LAST_RESULTS
    _ensure_axon_hooks_shim()
    from concourse import bass_utils

    B, T, _ = x.shape
    N = B * T
    xf = np.ascontiguousarray(x.reshape(N, C)).astype(np.float32, copy=False)

    # --- gate on host (f32, matches reference numerics) ---
    logits = xf @ np.ascontiguousarray(gate_w.astype(np.float32)).T
    m = logits.max(axis=1, keepdims=True)
    ew = np.exp(logits - m)
    sw = ew / ew.sum(axis=1, keepdims=True)        # [N, E] f32 softmax
    ar = np.arange(N)
    i0 = sw.argmax(axis=1)
    w0 = sw[ar, i0]
    swm = sw.copy()
    swm[ar, i0] = -1.0
    i1 = swm.argmax(axis=1)
    w1g = sw[ar, i1]
    tot = w0 + w1g
    cw0 = (w0 / tot).astype(np.float32)
    cw1 = (w1g / tot).astype(np.float32)

    # --- dispatch: token lists per expert ---
    idx_list, cw_list = [], []
    for e in range(E):
        s0 = i0 == e
        s1 = i1 == e
        idx_list.append(np.concatenate([ar[s0], ar[s1]]))
        cw_list.append(np.concatenate([cw0[s0], cw1[s1]]).astype(np.float32))
    counts = [len(ix) for ix in idx_list]
    cap = ((max(counts) + P - 1) // P) * P
    # equal-width chunks (multiples of 128, each <= 512): avoids a narrow
    # tail chunk where per-matmul LDWEIGHTS overhead is exposed
    n_chunks = -(-cap // 512)
    q, r = divmod(cap // P, n_chunks)
    chunks = [(q + 1) * P] * r + [q * P] * (n_chunks - r)

    nc = _get_compiled(cap, chunks)

    # --- per-core inputs ---
    w1b = w1.astype(BF16)                                        # [E, C, H]
    w2b = w2.astype(BF16)                                        # [E, H, C]
    b1r = np.ascontiguousarray(
        b1.astype(np.float32).reshape(E, KH, P).transpose(0, 2, 1))
    b2r = np.ascontiguousarray(
        b2.astype(np.float32).reshape(E, KC, P).transpose(0, 2, 1))
    in_maps = []
    for e in range(E):
        xt = np.zeros((C, cap), dtype=BF16)
        xt[:, :counts[e]] = np.ascontiguousarray(xf[idx_list[e]].T)
        in_maps.append({
            "xt": xt,
            "w1t": np.ascontiguousarray(w1b[e]),
            "w2t": np.ascontiguousarray(w2b[e]),
            "b1r": b1r[e],
            "b2r": b2r[e],
        })

    res = bass_utils.run_bass_kernel_spmd(
        nc, in_maps, core_ids=list(range(N_CORES)), trace=TRACE)
    LAST_RESULTS = res
    LAST_EXEC_NS = res.exec_time_ns

    # --- combine (host unshard) ---
    out = np.zeros((N, C), dtype=np.float32)
    for e in range(E):
        n_e = counts[e]
        y = res.results[e]["out"][:, :n_e].T                     # [n_e, C] f32
        out[idx_list[e]] += cw_list[e][:, None] * y
    return out.reshape(B, T, C).astype(x.dtype, copy=False)


# revision 9
# speedup vs baseline: 1.0797x; 1.0041x over previous
# MoE (top-2 of 8 experts) Trainium2 kernel.
#
# Strategy (expert-parallel, matches the sharding hint):
#   - Gate (softmax + top-2 + renormalize) computed on host in f32 — it is
#     0.006% of the FLOPs and produces the data-dependent routing needed to
#     shard the tokens.
#   - Token dispatch = the host-side sharding step: tokens routed to expert e
#     are gathered (transposed, bf16-cast, padded to a uniform capacity) and
#     sent to core e together with expert e's weights.
#   - Each core runs a dense FFN  relu(x @ w1 + b1) @ w2 + b2  over its token
#     batch on the TensorEngine (bf16 inputs, fp32 PSUM accumulation).
#   - Combine = host-side unshard: out[tok] += gate_weight * y_core[tok].
#
# Device kernel layout (all "transposed": tokens on the matmul free dim):
#   phase 1:  hT[mh]  = relu( w1[kc,:,mh*128:..].T @ xT[kc]  summed over kc + b1 )
#   phase 2:  outT[mc] =       w2[kh,:,mc*128:..].T @ hT[kh]  summed over kh + b2
# w1/w2 stay resident in SBUF (bf16, 16 MB); token chunks of 512 stream through.

import os
import sys
import types

import numpy as np
import ml_dtypes

P = 128
C = 1024
H = 4096
E = 8
N_CORES = 8
KC = C // P   # 8
KH = H // P   # 32
BF16 = ml_dtypes.bfloat16

TRACE = bool(int(os.environ.get("KERNEL_TRACE", "0")))
LAST_EXEC_NS = None
LAST_RESULTS = None


def _ensure_axon_hooks_shim():
    """bass_utils imports antenv.axon_hooks when tracing is requested; this
    image's antenv lacks that module. Provide it, backed by the axon PJRT .so
    profiling C ABI when available."""
    try:
        import antenv.axon_hooks  # noqa: F401
        return
    except ImportError:
        pass
    mod = types.ModuleType("antenv.axon_hooks")
    mod._hook = None

    def set_axon_ntff_profile_hook(h):
        mod._hook = h

    def get_axon_ntff_profile_hook():
        return mod._hook

    mod.set_axon_ntff_profile_hook = set_axon_ntff_profile_hook
    mod.get_axon_ntff_profile_hook = get_axon_ntff_profile_hook
    try:
        import antenv
        sys.modules["antenv.axon_hooks"] = mod
        antenv.axon_hooks = mod
    except ImportError:
        antenv = types.ModuleType("antenv")
        antenv.axon_hooks = mod
        sys.modules["antenv"] = antenv
        sys.modules["antenv.axon_hooks"] = mod
    try:
        from trn_agent_boot.trn_boot import _ntff_profile_via_ctypes
        h = _ntff_profile_via_ctypes("/opt/axon/libaxon_pjrt.so")
        if h is not None:
            mod._hook = h
    except Exception:
        pass


_COMPILED = {}


def _build(cap, chunks):
    import concourse.mybir as mybir
    import concourse.tile as tile
    from concourse import bacc

    f32 = mybir.dt.float32
    bf16 = mybir.dt.bfloat16

    nc = bacc.Bacc("TRN2", target_bir_lowering=False, debug=False,
                   num_devices=N_CORES)

    xt_d = nc.dram_tensor("xt", [C, cap], bf16, kind="ExternalInput")
    w1_d = nc.dram_tensor("w1t", [C, H], bf16, kind="ExternalInput")
    w2_d = nc.dram_tensor("w2t", [H, C], bf16, kind="ExternalInput")
    b1_d = nc.dram_tensor("b1r", [P, KH], f32, kind="ExternalInput")
    b2_d = nc.dram_tensor("b2r", [P, KC], f32, kind="ExternalInput")
    out_d = nc.dram_tensor("out", [C, cap], f32, kind="ExternalOutput")

    # partition-major views: [p, kc/kh/mc, free] so one DMA covers all
    # 128-row tiles of a tensor (each dma_start trigger costs ~600ns on the
    # Sync sequencer — merged transfers keep the trigger count tiny)
    xt_t = xt_d.ap().rearrange("(kc p) n -> p kc n", p=P)
    w1_t = w1_d.ap().rearrange("(kc p) h -> p kc h", p=P)
    w2_t = w2_d.ap().rearrange("(kh p) c -> p kh c", p=P)
    out_t = out_d.ap().rearrange("(mc p) n -> p mc n", p=P)

    relu = mybir.ActivationFunctionType.Relu

    # w1 is loaded in column blocks so the first phase-1 groups can start
    # after ~1 MB of DMA instead of waiting for all 16 MB of weights.
    W1BLK = 8                 # column blocks of w1
    W1BW = H // W1BLK         # 512 columns per block
    GRP_PER_BLK = W1BW // P   # 4 mh-groups per block

    with tile.TileContext(nc) as tc:
        with (
            tc.tile_pool(name="wres", bufs=1) as wpool,
            tc.tile_pool(name="bias", bufs=1) as bpool,
            tc.tile_pool(name="xin", bufs=1) as xpool,
            tc.tile_pool(name="hmid", bufs=1) as hpool,
            tc.tile_pool(name="oout", bufs=1) as opool,
            tc.tile_pool(name="ps1", bufs=4, space="PSUM") as ps1pool,
            tc.tile_pool(name="ps2", bufs=4, space="PSUM") as ps2pool,
        ):
            # biases + token chunks go over GpSimd's SWDGE queues so their
            # triggers run in parallel with the weight triggers on Sync.
            # b1 early: the phase-1 relu (which drains PSUM slots) needs it.
            b1_sb = bpool.tile([P, KH], f32, tag="b1")
            nc.gpsimd.dma_start(b1_sb[:], b1_d.ap())
            b2_sb = bpool.tile([P, KC], f32, tag="b2")
            nc.gpsimd.dma_start(b2_sb[:], b2_d.ap())

            # first token chunk: on the critical path
            W0 = chunks[0]
            x_first = xpool.tile([P, KC * W0], bf16, tag="x")
            nc.gpsimd.dma_start(
                x_first[:].rearrange("p (kc w) -> p kc w", kc=KC),
                xt_t[:, :, 0:W0])

            # w1: one DMA per column block, [p, kc, W1BW] layout in SBUF
            w1_sb = []
            for blk in range(W1BLK):
                t = wpool.tile([P, KC * W1BW], bf16, tag=f"w1_{blk}")
                nc.sync.dma_start(
                    t[:].rearrange("p (kc w) -> p kc w", kc=KC),
                    w1_t[:, :, blk * W1BW:(blk + 1) * W1BW])
                w1_sb.append(t)
            # w2: one DMA per output-column block, [p, kh, P] layout
            w2_sb = []
            for mc in range(KC):
                t = wpool.tile([P, KH * P], bf16, tag=f"w2_{mc}")
                nc.sync.dma_start(
                    t[:].rearrange("p (kh w) -> p kh w", kh=KH),
                    w2_t[:, :, mc * P:(mc + 1) * P])
                w2_sb.append(t)

            off = 0
            for ci, W in enumerate(chunks):
                if ci == 0:
                    x_sb = x_first
                else:
                    x_sb = xpool.tile([P, KC * W], bf16, tag="x")
                    nc.gpsimd.dma_start(
                        x_sb[:].rearrange("p (kc w) -> p kc w", kc=KC),
                        xt_t[:, :, off:off + W])

                h_sb = []
                for mh in range(KH):
                    blk, col = divmod(mh, GRP_PER_BLK)
                    ps = ps1pool.tile([P, W], f32, tag="ps1")
                    for kc in range(KC):
                        nc.tensor.matmul(
                            ps[:],
                            w1_sb[blk][:, (kc * GRP_PER_BLK + col) * P:
                                       (kc * GRP_PER_BLK + col) * P + P],
                            x_sb[:, kc * W:(kc + 1) * W],
                            start=(kc == 0),
                            stop=(kc == KC - 1),
                        )
                    ht = hpool.tile([P, W], bf16, tag=f"h_{mh}")
                    nc.scalar.activation(ht[:], ps[:], relu,
                                         bias=b1_sb[:, mh:mh + 1], scale=1.0)
                    h_sb.append(ht)

                # last chunk: two half-tiles so the first half's store drains
                # while the last PSUM groups finish — shorter kernel tail
                n_osplit = 2 if ci == len(chunks) - 1 else 1
                mc_per = KC // n_osplit
                for s in range(n_osplit):
                    o_sb = opool.tile([P, mc_per * W], f32, tag=f"o_{s}")
                    for mci in range(mc_per):
                        mc = s * mc_per + mci
                        ps = ps2pool.tile([P, W], f32, tag="ps2")
                        for kh in range(KH):
                            nc.tensor.matmul(
                                ps[:],
                                w2_sb[mc][:, kh * P:(kh + 1) * P],
                                h_sb[kh][:],
                                start=(kh == 0),
                                stop=(kh == KH - 1),
                            )
                        nc.vector.tensor_scalar_add(
                            o_sb[:, mci * W:(mci + 1) * W], ps[:],
                            b2_sb[:, mc:mc + 1])
                    nc.sync.dma_start(
                        out_t[:, s * mc_per:(s + 1) * mc_per, off:off + W],
                        o_sb[:].rearrange("p (mc w) -> p mc w", mc=mc_per))
                off += W

    nc.compile()
    return nc


def _get_compiled(cap, chunks):
    key = (cap, tuple(chunks))
    if key not in _COMPILED:
        _COMPILED[key] = _build(cap, list(chunks))
    return _COMPILED[key]


def kernel(x, gate_w, w1, b1, w2, b2):
    global LAST_EXEC_NS, LAST_RESULTS
    _ensure_axon_hooks_shim()
    from concourse import bass_utils

    B, T, _ = x.shape
    N = B * T
    xf = np.ascontiguousarray(x.reshape(N, C)).astype(np.float32, copy=False)

    # --- gate on host (f32, matches reference numerics) ---
    logits = xf @ np.ascontiguousarray(gate_w.astype(np.float32)).T
    m = logits.max(axis=1, keepdims=True)
    ew = np.exp(logits - m)
    sw = ew / ew.sum(axis=1, keepdims=True)        # [N, E] f32 softmax
    ar = np.arange(N)
    i0 = sw.argmax(axis=1)
    w0 = sw[ar, i0]
    swm = sw.copy()
    swm[ar, i0] = -1.0
    i1 = swm.argmax(axis=1)
    w1g = sw[ar, i1]
    tot = w0 + w1g
    cw0 = (w0 / tot).astype(np.float32)
    cw1 = (w1g / tot).astype(np.float32)

    # --- dispatch: token lists per expert ---
    idx_list, cw_list = [], []
    for e in range(E):
        s0 = i0 == e
        s1 = i1 == e
        idx_list.append(np.concatenate([ar[s0], ar[s1]]))
        cw_list.append(np.concatenate([cw0[s0], cw1[s1]]).astype(np.float32))
    counts = [len(ix) for ix in idx_list]
    cap = ((max(counts) + P - 1) // P) * P
    # equal-width chunks (multiples of 128, each <= 512): avoids a narrow
    # tail chunk where per-matmul LDWEIGHTS overhead is exposed
    n_chunks = -(-cap // 512)
    q, r = divmod(cap // P, n_chunks)
    chunks = [(q + 1) * P] * r + [q * P] * (n_chunks - r)

    nc = _get_compiled(cap, chunks)

    # --- per-core inputs ---
    w1b = w1.astype(BF16)                                        # [E, C, H]
    w2b = w2.astype(BF16)                                        # [E, H, C]
    b1r = np.ascontiguousarray(
        b1.astype(np.float32).reshape(E, KH, P).transpose(0, 2, 1))
    b2r = np.ascontiguousarray(
        b2.astype(np.float32).reshape(E, KC, P).transpose(0, 2, 1))
    in_maps = []
    for e in range(E):
        xt = np.zeros((C, cap), dtype=BF16)
        xt[:, :counts[e]] = np.ascontiguousarray(xf[idx_list[e]].T)
        in_maps.append({
            "xt": xt,
            "w1t": np.ascontiguousarray(w1b[e]),
            "w2t": np.ascontiguousarray(w2b[e]),
            "b1r": b1r[e],
            "b2r": b2r[e],
        })

    res = bass_utils.run_bass_kernel_spmd(
        nc, in_maps, core_ids=list(range(N_CORES)), trace=TRACE)
    LAST_RESULTS = res
    LAST_EXEC_NS = res.exec_time_ns

    # --- combine (host unshard) ---
    out = np.zeros((N, C), dtype=np.float32)
    for e in range(E):
        n_e = counts[e]
        y = res.results[e]["out"][:, :n_e].T                     # [n_e, C] f32
        out[idx_list[e]] += cw_list[e][:, None] * y
    return out.reshape(B, T, C).astype(x.dtype, copy=False)


# revision 10
# speedup vs baseline: 1.0878x; 1.0076x over previous
# MoE (top-2 of 8 experts) Trainium2 kernel.
#
# Strategy (expert-parallel, matches the sharding hint):
#   - Gate (softmax + top-2 + renormalize) computed on host in f32 — it is
#     0.006% of the FLOPs and produces the data-dependent routing needed to
#     shard the tokens.
#   - Token dispatch = the host-side sharding step: tokens routed to expert e
#     are gathered (transposed, bf16-cast, padded to a uniform capacity) and
#     sent to core e together with expert e's weights.
#   - Each core runs a dense FFN  relu(x @ w1 + b1) @ w2 + b2  over its token
#     batch on the TensorEngine (bf16 inputs, fp32 PSUM accumulation).
#   - Combine = host-side unshard: out[tok] += gate_weight * y_core[tok].
#
# Device kernel layout (all "transposed": tokens on the matmul free dim):
#   phase 1:  hT[mh]  = relu( w1[kc,:,mh*128:..].T @ xT[kc]  summed over kc + b1 )
#   phase 2:  outT[mc] =       w2[kh,:,mc*128:..].T @ hT[kh]  summed over kh + b2
# w1/w2 stay resident in SBUF (bf16, 16 MB); token chunks of 512 stream through.

import os
import sys
import types

import numpy as np
import ml_dtypes

P = 128
C = 1024
H = 4096
E = 8
N_CORES = 8
KC = C // P   # 8
KH = H // P   # 32
BF16 = ml_dtypes.bfloat16

TRACE = bool(int(os.environ.get("KERNEL_TRACE", "0")))
LAST_EXEC_NS = None
LAST_RESULTS = None


def _ensure_axon_hooks_shim():
    """bass_utils imports antenv.axon_hooks when tracing is requested; this
    image's antenv lacks that module. Provide it, backed by the axon PJRT .so
    profiling C ABI when available."""
    try:
        import antenv.axon_hooks  # noqa: F401
        return
    except ImportError:
        pass
    mod = types.ModuleType("antenv.axon_hooks")
    mod._hook = None

    def set_axon_ntff_profile_hook(h):
        mod._hook = h

    def get_axon_ntff_profile_hook():
        return mod._hook

    mod.set_axon_ntff_profile_hook = set_axon_ntff_profile_hook
    mod.get_axon_ntff_profile_hook = get_axon_ntff_profile_hook
    try:
        import antenv
        sys.modules["antenv.axon_hooks"] = mod
        antenv.axon_hooks = mod
    except ImportError:
        antenv = types.ModuleType("antenv")
        antenv.axon_hooks = mod
        sys.modules["antenv"] = antenv
        sys.modules["antenv.axon_hooks"] = mod
    try:
        from trn_agent_boot.trn_boot import _ntff_profile_via_ctypes
        h = _ntff_profile_via_ctypes("/opt/axon/libaxon_pjrt.so")
        if h is not None:
            mod._hook = h
    except Exception:
        pass


_COMPILED = {}


def _build(cap, chunks):
    import concourse.mybir as mybir
    import concourse.tile as tile
    from concourse import bacc

    f32 = mybir.dt.float32
    bf16 = mybir.dt.bfloat16

    nc = bacc.Bacc("TRN2", target_bir_lowering=False, debug=False,
                   num_devices=N_CORES)

    xt_d = nc.dram_tensor("xt", [C, cap], bf16, kind="ExternalInput")
    w1_d = nc.dram_tensor("w1t", [C, H], bf16, kind="ExternalInput")
    w2_d = nc.dram_tensor("w2t", [H, C], bf16, kind="ExternalInput")
    b1_d = nc.dram_tensor("b1r", [P, KH], f32, kind="ExternalInput")
    b2_d = nc.dram_tensor("b2r", [P, KC], f32, kind="ExternalInput")
    out_d = nc.dram_tensor("out", [C, cap], f32, kind="ExternalOutput")

    # partition-major views: [p, kc/kh/mc, free] so one DMA covers all
    # 128-row tiles of a tensor (each dma_start trigger costs ~600ns on the
    # Sync sequencer — merged transfers keep the trigger count tiny)
    xt_t = xt_d.ap().rearrange("(kc p) n -> p kc n", p=P)
    w1_t = w1_d.ap().rearrange("(kc p) h -> p kc h", p=P)
    w2_t = w2_d.ap().rearrange("(kh p) c -> p kh c", p=P)
    out_t = out_d.ap().rearrange("(mc p) n -> p mc n", p=P)

    relu = mybir.ActivationFunctionType.Relu

    # w1 is loaded in column blocks so the first phase-1 groups can start
    # after ~1 MB of DMA instead of waiting for all 16 MB of weights.
    W1BLK = 8                 # column blocks of w1
    W1BW = H // W1BLK         # 512 columns per block
    GRP_PER_BLK = W1BW // P   # 4 mh-groups per block

    with tile.TileContext(nc) as tc:
        with (
            tc.tile_pool(name="wres", bufs=1) as wpool,
            tc.tile_pool(name="bias", bufs=1) as bpool,
            tc.tile_pool(name="xin", bufs=1) as xpool,
            tc.tile_pool(name="hmid", bufs=1) as hpool,
            tc.tile_pool(name="oout", bufs=1) as opool,
            tc.tile_pool(name="ps1", bufs=4, space="PSUM") as ps1pool,
            tc.tile_pool(name="ps2", bufs=4, space="PSUM") as ps2pool,
        ):
            # biases + token chunks go over GpSimd's SWDGE queues so their
            # triggers run in parallel with the weight triggers on Sync.
            # b1 early: the phase-1 relu (which drains PSUM slots) needs it.
            b1_sb = bpool.tile([P, KH], f32, tag="b1")
            nc.gpsimd.dma_start(b1_sb[:], b1_d.ap())
            b2_sb = bpool.tile([P, KC], f32, tag="b2")
            nc.gpsimd.dma_start(b2_sb[:], b2_d.ap())

            # first token chunk: on the critical path, so it goes on Sync
            # (HWDGE) ahead of the weights — SWDGE descriptor generation is
            # too slow for this 3D pattern
            W0 = chunks[0]
            x_first = xpool.tile([P, KC * W0], bf16, tag="x")
            nc.sync.dma_start(
                x_first[:].rearrange("p (kc w) -> p kc w", kc=KC),
                xt_t[:, :, 0:W0])

            # w1: one DMA per column block, [p, kc, W1BW] layout in SBUF
            w1_sb = []
            for blk in range(W1BLK):
                t = wpool.tile([P, KC * W1BW], bf16, tag=f"w1_{blk}")
                nc.sync.dma_start(
                    t[:].rearrange("p (kc w) -> p kc w", kc=KC),
                    w1_t[:, :, blk * W1BW:(blk + 1) * W1BW])
                w1_sb.append(t)
            # w2: one DMA per output-column block, [p, kh, P] layout
            w2_sb = []
            for mc in range(KC):
                t = wpool.tile([P, KH * P], bf16, tag=f"w2_{mc}")
                nc.sync.dma_start(
                    t[:].rearrange("p (kh w) -> p kh w", kh=KH),
                    w2_t[:, :, mc * P:(mc + 1) * P])
                w2_sb.append(t)

            off = 0
            for ci, W in enumerate(chunks):
                if ci == 0:
                    x_sb = x_first
                else:
                    x_sb = xpool.tile([P, KC * W], bf16, tag="x")
                    nc.gpsimd.dma_start(
                        x_sb[:].rearrange("p (kc w) -> p kc w", kc=KC),
                        xt_t[:, :, off:off + W])

                h_sb = []
                for mh in range(KH):
                    blk, col = divmod(mh, GRP_PER_BLK)
                    ps = ps1pool.tile([P, W], f32, tag="ps1")
                    for kc in range(KC):
                        nc.tensor.matmul(
                            ps[:],
                            w1_sb[blk][:, (kc * GRP_PER_BLK + col) * P:
                                       (kc * GRP_PER_BLK + col) * P + P],
                            x_sb[:, kc * W:(kc + 1) * W],
                            start=(kc == 0),
                            stop=(kc == KC - 1),
                        )
                    ht = hpool.tile([P, W], bf16, tag=f"h_{mh}")
                    nc.scalar.activation(ht[:], ps[:], relu,
                                         bias=b1_sb[:, mh:mh + 1], scale=1.0)
                    h_sb.append(ht)

                # last chunk: two half-tiles so the first half's store drains
                # while the last PSUM groups finish — shorter kernel tail
                n_osplit = 2 if ci == len(chunks) - 1 else 1
                mc_per = KC // n_osplit
                for s in range(n_osplit):
                    o_sb = opool.tile([P, mc_per * W], f32, tag=f"o_{s}")
                    for mci in range(mc_per):
                        mc = s * mc_per + mci
                        ps = ps2pool.tile([P, W], f32, tag="ps2")
                        for kh in range(KH):
                            nc.tensor.matmul(
                                ps[:],
                                w2_sb[mc][:, kh * P:(kh + 1) * P],
                                h_sb[kh][:],
                                start=(kh == 0),
                                stop=(kh == KH - 1),
                            )
                        nc.vector.tensor_scalar_add(
                            o_sb[:, mci * W:(mci + 1) * W], ps[:],
                            b2_sb[:, mc:mc + 1])
                    nc.sync.dma_start(
                        out_t[:, s * mc_per:(s + 1) * mc_per, off:off + W],
                        o_sb[:].rearrange("p (mc w) -> p mc w", mc=mc_per))
                off += W

    nc.compile()
    return nc


def _get_compiled(cap, chunks):
    key = (cap, tuple(chunks))
    if key not in _COMPILED:
        _COMPILED[key] = _build(cap, list(chunks))
    return _COMPILED[key]


def kernel(x, gate_w, w1, b1, w2, b2):
    global LAST_EXEC_NS, LAST_RESULTS
    _ensure_axon_hooks_shim()
    from concourse import bass_utils

    B, T, _ = x.shape
    N = B * T
    xf = np.ascontiguousarray(x.reshape(N, C)).astype(np.float32, copy=False)

    # --- gate on host (f32, matches reference numerics) ---
    logits = xf @ np.ascontiguousarray(gate_w.astype(np.float32)).T
    m = logits.max(axis=1, keepdims=True)
    ew = np.exp(logits - m)
    sw = ew / ew.sum(axis=1, keepdims=True)        # [N, E] f32 softmax
    ar = np.arange(N)
    i0 = sw.argmax(axis=1)
    w0 = sw[ar, i0]
    swm = sw.copy()
    swm[ar, i0] = -1.0
    i1 = swm.argmax(axis=1)
    w1g = sw[ar, i1]
    tot = w0 + w1g
    cw0 = (w0 / tot).astype(np.float32)
    cw1 = (w1g / tot).astype(np.float32)

    # --- dispatch: token lists per expert ---
    idx_list, cw_list = [], []
    for e in range(E):
        s0 = i0 == e
        s1 = i1 == e
        idx_list.append(np.concatenate([ar[s0], ar[s1]]))
        cw_list.append(np.concatenate([cw0[s0], cw1[s1]]).astype(np.float32))
    counts = [len(ix) for ix in idx_list]
    cap = ((max(counts) + P - 1) // P) * P
    # equal-width chunks (multiples of 128, each <= 512): avoids a narrow
    # tail chunk where per-matmul LDWEIGHTS overhead is exposed
    n_chunks = -(-cap // 512)
    q, r = divmod(cap // P, n_chunks)
    chunks = [(q + 1) * P] * r + [q * P] * (n_chunks - r)

    nc = _get_compiled(cap, chunks)

    # --- per-core inputs ---
    w1b = w1.astype(BF16)                                        # [E, C, H]
    w2b = w2.astype(BF16)                                        # [E, H, C]
    b1r = np.ascontiguousarray(
        b1.astype(np.float32).reshape(E, KH, P).transpose(0, 2, 1))
    b2r = np.ascontiguousarray(
        b2.astype(np.float32).reshape(E, KC, P).transpose(0, 2, 1))
    in_maps = []
    for e in range(E):
        xt = np.zeros((C, cap), dtype=BF16)
        xt[:, :counts[e]] = np.ascontiguousarray(xf[idx_list[e]].T)
        in_maps.append({
            "xt": xt,
            "w1t": np.ascontiguousarray(w1b[e]),
            "w2t": np.ascontiguousarray(w2b[e]),
            "b1r": b1r[e],
            "b2r": b2r[e],
        })

    res = bass_utils.run_bass_kernel_spmd(
        nc, in_maps, core_ids=list(range(N_CORES)), trace=TRACE)
    LAST_RESULTS = res
    LAST_EXEC_NS = res.exec_time_ns

    # --- combine (host unshard) ---
    out = np.zeros((N, C), dtype=np.float32)
    for e in range(E):
        n_e = counts[e]
        y = res.results[e]["out"][:, :n_e].T                     # [n_e, C] f32
        out[idx_list[e]] += cw_list[e][:, None] * y
    return out.reshape(B, T, C).astype(x.dtype, copy=False)


# revision 11
# speedup vs baseline: 1.1077x; 1.0183x over previous
# MoE (top-2 of 8 experts) Trainium2 kernel.
#
# Strategy — expert-parallel with pairwise H-split (a refinement of the
# "shard experts across devices, all-to-all dispatch" hint):
#   - Gate (softmax + top-2 + renormalize) computed on host in f32 — it is
#     0.006% of the FLOPs and produces the data-dependent routing needed to
#     shard the tokens.
#   - Experts are sorted by routed-token count: the 4 heaviest and the 4
#     lightest are paired up. Each of the 4 pairs maps onto 2 cores: both
#     cores process BOTH experts' full token batches, but each core computes
#     only half of the hidden dimension H. The host sums the two half-H
#     partials. This caps per-core work at (max_heavy + max_light)/2 token
#     FFNs instead of max_all, which is much closer to the perfect 2048.
#   - Device per token batch: dense FFN  relu(x @ w1h + b1h) @ w2h on the
#     TensorEngine (bf16 inputs, fp32 PSUM accumulation); b2 and the gate
#     combine weights are applied on the host during unshard.
#
# Device kernel layout (all "transposed": tokens on the matmul free dim):
#   phase 1:  hT[mh]  = relu( w1[kc,:,mh*128:..].T @ xT[kc]  summed over kc + b1 )
#   phase 2:  outT[mc] =       w2[kh,:,mc*128:..].T @ hT[kh]  summed over kh
# Both experts' half-H weights stay resident in SBUF (bf16, 16 MB); token
# chunks of <=512 stream through.

import os
import sys
import types

import numpy as np
import ml_dtypes

P = 128
C = 1024
H = 4096
H2 = H // 2
E = 8
N_CORES = 8
KC = C // P     # 8
KH2 = H2 // P   # 16
BF16 = ml_dtypes.bfloat16

TRACE = bool(int(os.environ.get("KERNEL_TRACE", "0")))
LAST_EXEC_NS = None
LAST_RESULTS = None


def _ensure_axon_hooks_shim():
    """bass_utils imports antenv.axon_hooks when tracing is requested; this
    image's antenv lacks that module. Provide it, backed by the axon PJRT .so
    profiling C ABI when available."""
    try:
        import antenv.axon_hooks  # noqa: F401
        return
    except ImportError:
        pass
    mod = types.ModuleType("antenv.axon_hooks")
    mod._hook = None

    def set_axon_ntff_profile_hook(h):
        mod._hook = h

    def get_axon_ntff_profile_hook():
        return mod._hook

    mod.set_axon_ntff_profile_hook = set_axon_ntff_profile_hook
    mod.get_axon_ntff_profile_hook = get_axon_ntff_profile_hook
    try:
        import antenv
        sys.modules["antenv.axon_hooks"] = mod
        antenv.axon_hooks = mod
    except ImportError:
        antenv = types.ModuleType("antenv")
        antenv.axon_hooks = mod
        sys.modules["antenv"] = antenv
        sys.modules["antenv.axon_hooks"] = mod
    try:
        from trn_agent_boot.trn_boot import _ntff_profile_via_ctypes
        h = _ntff_profile_via_ctypes("/opt/axon/libaxon_pjrt.so")
        if h is not None:
            mod._hook = h
    except Exception:
        pass


_COMPILED = {}


def _equal_chunks(cap):
    n = -(-cap // 512)
    q, r = divmod(cap // P, n)
    return [(q + 1) * P] * r + [q * P] * (n - r)


def _build(cap_a, cap_b):
    import concourse.mybir as mybir
    import concourse.tile as tile
    from concourse import bacc

    f32 = mybir.dt.float32
    bf16 = mybir.dt.bfloat16

    nc = bacc.Bacc("TRN2", target_bir_lowering=False, debug=False,
                   num_devices=N_CORES)

    caps = {"a": cap_a, "b": cap_b}
    x_d, w1_d, w2_d, out_d = {}, {}, {}, {}
    for s in ("a", "b"):
        x_d[s] = nc.dram_tensor(f"xt{s}", [C, caps[s]], bf16,
                                kind="ExternalInput")
        w1_d[s] = nc.dram_tensor(f"w1{s}", [C, H2], bf16,
                                 kind="ExternalInput")
        w2_d[s] = nc.dram_tensor(f"w2{s}", [H2, C], bf16,
                                 kind="ExternalInput")
        out_d[s] = nc.dram_tensor(f"out{s}", [C, caps[s]], f32,
                                  kind="ExternalOutput")
    b1_d = nc.dram_tensor("b1r", [P, 2 * KH2], f32, kind="ExternalInput")

    # partition-major views: [p, kc/kh/mc, free] so one DMA covers all
    # 128-row tiles of a tensor (each dma_start trigger costs ~600ns on the
    # Sync sequencer — merged transfers keep the trigger count tiny)
    x_t = {s: x_d[s].ap().rearrange("(kc p) n -> p kc n", p=P)
           for s in ("a", "b")}
    w1_t = {s: w1_d[s].ap().rearrange("(kc p) h -> p kc h", p=P)
            for s in ("a", "b")}
    w2_t = {s: w2_d[s].ap().rearrange("(kh p) c -> p kh c", p=P)
            for s in ("a", "b")}
    out_t = {s: out_d[s].ap().rearrange("(mc p) n -> p mc n", p=P)
             for s in ("a", "b")}

    relu = mybir.ActivationFunctionType.Relu

    # w1 loaded in column blocks so the first phase-1 groups can start after
    # ~1 MB of DMA instead of waiting for all 16 MB of weights
    W1BW = 512               # columns per w1 block
    W1BLK = H2 // W1BW       # 4 blocks per segment
    GRP_PER_BLK = W1BW // P  # 4 mh-groups per block

    chunks = {s: _equal_chunks(caps[s]) for s in ("a", "b")}

    with tile.TileContext(nc) as tc:
        with (
            tc.tile_pool(name="wres", bufs=1) as wpool,
            tc.tile_pool(name="bias", bufs=1) as bpool,
            tc.tile_pool(name="xin", bufs=2) as xpool,
            tc.tile_pool(name="hmid", bufs=1) as hpool,
            tc.tile_pool(name="oout", bufs=1) as opool,
            tc.tile_pool(name="ps1", bufs=4, space="PSUM") as ps1pool,
            tc.tile_pool(name="ps2", bufs=4, space="PSUM") as ps2pool,
        ):
            # bias via GpSimd SWDGE: its trigger runs in parallel with the
            # Sync-side loads; the phase-1 relu (which drains PSUM slots)
            # needs b1 early
            b1_sb = bpool.tile([P, 2 * KH2], f32, tag="b1")
            nc.gpsimd.dma_start(b1_sb[:], b1_d.ap())

            # first token chunk of segment a: on the critical path
            W0 = chunks["a"][0]
            x_first = xpool.tile([P, KC * W0], bf16, tag="x")
            nc.sync.dma_start(
                x_first[:].rearrange("p (kc w) -> p kc w", kc=KC),
                x_t["a"][:, :, 0:W0])

            # weights in consumption order: w1a, w2a, w1b, w2b
            w1_sb, w2_sb = {}, {}
            for s in ("a", "b"):
                w1_sb[s] = []
                for blk in range(W1BLK):
                    t = wpool.tile([P, KC * W1BW], bf16, tag=f"w1{s}_{blk}")
                    nc.sync.dma_start(
                        t[:].rearrange("p (kc w) -> p kc w", kc=KC),
                        w1_t[s][:, :, blk * W1BW:(blk + 1) * W1BW])
                    w1_sb[s].append(t)
                w2_sb[s] = []
                for mc in range(KC):
                    t = wpool.tile([P, KH2 * P], bf16, tag=f"w2{s}_{mc}")
                    nc.sync.dma_start(
                        t[:].rearrange("p (kh w) -> p kh w", kh=KH2),
                        w2_t[s][:, :, mc * P:(mc + 1) * P])
                    w2_sb[s].append(t)

            for si, s in enumerate(("a", "b")):
                b1_off = si * KH2
                off = 0
                seg_chunks = chunks[s]
                for ci, W in enumerate(seg_chunks):
                    if si == 0 and ci == 0:
                        x_sb = x_first
                    else:
                        x_sb = xpool.tile([P, KC * W], bf16, tag="x")
                        nc.gpsimd.dma_start(
                            x_sb[:].rearrange("p (kc w) -> p kc w", kc=KC),
                            x_t[s][:, :, off:off + W])

                    h_sb = []
                    for mh in range(KH2):
                        blk, col = divmod(mh, GRP_PER_BLK)
                        ps = ps1pool.tile([P, W], f32, tag="ps1")
                        for kc in range(KC):
                            nc.tensor.matmul(
                                ps[:],
                                w1_sb[s][blk][:, (kc * GRP_PER_BLK + col) * P:
                                              (kc * GRP_PER_BLK + col) * P + P],
                                x_sb[:, kc * W:(kc + 1) * W],
                                start=(kc == 0),
                                stop=(kc == KC - 1),
                            )
                        ht = hpool.tile([P, W], bf16, tag=f"h_{mh}")
                        nc.scalar.activation(
                            ht[:], ps[:], relu,
                            bias=b1_sb[:, b1_off + mh:b1_off + mh + 1],
                            scale=1.0)
                        h_sb.append(ht)

                    # last chunk overall: two half-tiles so the first half's
                    # store drains while the last PSUM groups finish
                    last = (si == 1 and ci == len(seg_chunks) - 1)
                    n_osplit = 2 if last else 1
                    mc_per = KC // n_osplit
                    for sp in range(n_osplit):
                        o_sb = opool.tile([P, mc_per * W], f32, tag=f"o_{sp}")
                        for mci in range(mc_per):
                            mc = sp * mc_per + mci
                            ps = ps2pool.tile([P, W], f32, tag="ps2")
                            for kh in range(KH2):
                                nc.tensor.matmul(
                                    ps[:],
                                    w2_sb[s][mc][:, kh * P:(kh + 1) * P],
                                    h_sb[kh][:],
                                    start=(kh == 0),
                                    stop=(kh == KH2 - 1),
                                )
                            nc.vector.tensor_copy(
                                o_sb[:, mci * W:(mci + 1) * W], ps[:])
                        nc.sync.dma_start(
                            out_t[s][:, sp * mc_per:(sp + 1) * mc_per,
                                     off:off + W],
                            o_sb[:].rearrange("p (mc w) -> p mc w", mc=mc_per))
                    off += W

    nc.compile()
    return nc


def _get_compiled(cap_a, cap_b):
    key = (cap_a, cap_b)
    if key not in _COMPILED:
        _COMPILED[key] = _build(cap_a, cap_b)
    return _COMPILED[key]


def kernel(x, gate_w, w1, b1, w2, b2):
    global LAST_EXEC_NS, LAST_RESULTS
    _ensure_axon_hooks_shim()
    from concourse import bass_utils

    B, T, _ = x.shape
    N = B * T
    xf = np.ascontiguousarray(x.reshape(N, C)).astype(np.float32, copy=False)

    # --- gate on host (f32, matches reference numerics) ---
    logits = xf @ np.ascontiguousarray(gate_w.astype(np.float32)).T
    m = logits.max(axis=1, keepdims=True)
    ew = np.exp(logits - m)
    sw = ew / ew.sum(axis=1, keepdims=True)        # [N, E] f32 softmax
    ar = np.arange(N)
    i0 = sw.argmax(axis=1)
    w0 = sw[ar, i0]
    swm = sw.copy()
    swm[ar, i0] = -1.0
    i1 = swm.argmax(axis=1)
    w1g = sw[ar, i1]
    tot = w0 + w1g
    cw0 = (w0 / tot).astype(np.float32)
    cw1 = (w1g / tot).astype(np.float32)

    # --- dispatch: token lists per expert ---
    idx_list, cw_list = [], []
    for e in range(E):
        s0 = i0 == e
        s1 = i1 == e
        idx_list.append(np.concatenate([ar[s0], ar[s1]]))
        cw_list.append(np.concatenate([cw0[s0], cw1[s1]]).astype(np.float32))
    counts = np.array([len(ix) for ix in idx_list])

    # pair heavy experts with light ones; each pair -> 2 cores (H halves)
    order = np.argsort(-counts, kind="stable")
    big4, small4 = order[:4], order[4:]
    cap_a = max(((counts[big4].max() + P - 1) // P) * P, P)
    cap_b = max(((counts[small4].max() + P - 1) // P) * P, P)

    nc = _get_compiled(int(cap_a), int(cap_b))

    # --- per-core inputs ---
    w1b16 = w1.astype(BF16)                                      # [E, C, H]
    w2b16 = w2.astype(BF16)                                      # [E, H, C]
    b1f = b1.astype(np.float32)

    def xt_for(e, cap):
        xt = np.zeros((C, cap), dtype=BF16)
        xt[:, :counts[e]] = np.ascontiguousarray(xf[idx_list[e]].T)
        return xt

    xta = {int(e): xt_for(int(e), int(cap_a)) for e in big4}
    xtb = {int(e): xt_for(int(e), int(cap_b)) for e in small4}

    in_maps = []
    for core in range(N_CORES):
        i, h = divmod(core, 2)
        ea, eb = int(big4[i]), int(small4[i])
        hs = slice(h * H2, (h + 1) * H2)
        b1r = np.concatenate([
            b1f[ea, hs].reshape(KH2, P).T,
            b1f[eb, hs].reshape(KH2, P).T,
        ], axis=1)
        in_maps.append({
            "xta": xta[ea],
            "xtb": xtb[eb],
            "w1a": np.ascontiguousarray(w1b16[ea][:, hs]),
            "w1b": np.ascontiguousarray(w1b16[eb][:, hs]),
            "w2a": np.ascontiguousarray(w2b16[ea][hs, :]),
            "w2b": np.ascontiguousarray(w2b16[eb][hs, :]),
            "b1r": np.ascontiguousarray(b1r),
        })

    res = bass_utils.run_bass_kernel_spmd(
        nc, in_maps, core_ids=list(range(N_CORES)), trace=TRACE)
    LAST_RESULTS = res
    LAST_EXEC_NS = res.exec_time_ns

    # --- combine (host unshard): sum H-halves, add b2, apply gate weights ---
    out = np.zeros((N, C), dtype=np.float32)
    b2f = b2.astype(np.float32)
    for i in range(4):
        for seg, e_arr in (("a", big4), ("b", small4)):
            e = int(e_arr[i])
            n_e = int(counts[e])
            y = (res.results[2 * i][f"out{seg}"][:, :n_e].T +
                 res.results[2 * i + 1][f"out{seg}"][:, :n_e].T)
            y += b2f[e][None, :]
            out[idx_list[e]] += cw_list[e][:, None] * y
    return out.reshape(B, T, C).astype(x.dtype, copy=False)
